# revision 5
# baseline (speedup 1.0000x reference)
"""Bahdanau (additive) attention kernel for Trainium2, 8 NeuronCores.

Full-input contract: kernel(**inputs) takes the unsharded numpy inputs and
returns the full [TQ, B, D] output. Internally shards (batch, query-half)
across 8 cores (B=4 x 2 halves of Tq), runs a Bass/Tile kernel per core via
run_bass_kernel_spmd, and reassembles.

Math per core (b = batch, 128 local queries):
  wqT[u,q] = sum_d W1[d,u] q[q,d]          (PE matmul, fp32)
  wkT[u,v] = sum_d W2[d,u] v[v,d]          (PE matmul, fp32; kept in PSUM)
  g_q[u,v] = tanh(wkT[u,v] + wqT[u,q])     (ACT, per-partition bias = wqT[:,q])
  scores[q,v] = sum_u scale[u] g_q[u,v]    (PE matmul, float32r, sliding-window
                                            lhsT = sigma in column q, else 0)
  scores += maskadd[v]                     (PE K=1 matmul, ones lhsT)
  e = exp(scores); ssum = rowsum(e)        (ACT with accum_out)
  ctx[q,d] = (1/ssum[q]) sum_v e[q,v] v[v,d]  (PE transpose + matmuls, DVE scale)
"""

import sys

if "/opt/trn_rl_repo" not in sys.path:
    sys.path.insert(0, "/opt/trn_rl_repo")

import numpy as np

TQ, TV, B, D, U = 256, 1024, 4, 128, 128
NCORES = 8
TQL = 128  # local queries per core (Tq=256 split in 2 per batch)
NEG_INF = -1e9

# Score-contraction matmul dtype: "f32r" (1 cyc/row, reduced precision) or
# "f32" (exact, 4 cyc/row -> PE becomes the bottleneck).
SCORE_DT = "f32r"

_CACHE = {}


def _build_nc():
    import concourse.bacc as bacc
    import concourse.mybir as mybir
    import concourse.tile as tile
    from contextlib import ExitStack

    f32 = mybir.dt.float32
    f32r = mybir.dt.float32r
    AFT = mybir.ActivationFunctionType

    nc = bacc.Bacc("TRN2", target_bir_lowering=False, debug=False,
                   num_devices=NCORES)

    qt = nc.dram_tensor("qt", [D, TQL], f32, kind="ExternalInput").ap()
    vt = nc.dram_tensor("vt", [D, TV], f32, kind="ExternalInput").ap()
    vnp = nc.dram_tensor("vnp", [128, TV], f32, kind="ExternalInput").ap()
    w1 = nc.dram_tensor("w1", [D, U], f32, kind="ExternalInput").ap()
    w2 = nc.dram_tensor("w2", [D, U], f32, kind="ExternalInput").ap()
    sige = nc.dram_tensor("sige", [U, 2 * TQL - 1],
                          f32r if SCORE_DT == "f32r" else f32,
                          kind="ExternalInput").ap()
    mka = nc.dram_tensor("mka", [1, TV], f32, kind="ExternalInput").ap()
    ones1 = nc.dram_tensor("ones1", [1, TQL], f32, kind="ExternalInput").ap()
    ident = nc.dram_tensor("ident", [128, 128], f32, kind="ExternalInput").ap()
    out = nc.dram_tensor("out", [TQL, D], f32, kind="ExternalOutput").ap()

    NVC = TV // 128   # 8 value chunks of 128
    NJ = TV // 512    # 2 matmul slices of 512 (PSUM-bank limit)

    # fp32r operands must be *produced* as float32r (BIR verifier requires the
    # producer to round) — so the g tile and sigma tile are natively f32r.
    sdt = f32r if SCORE_DT == "f32r" else f32

    with tile.TileContext(nc) as tc:
        with ExitStack() as ctx:
            consts = ctx.enter_context(tc.tile_pool(name="consts", bufs=1))
            gpool = ctx.enter_context(tc.tile_pool(name="g", bufs=4))
            smp = ctx.enter_context(tc.tile_pool(name="sm", bufs=1))
            etp = ctx.enter_context(tc.tile_pool(name="et", bufs=2))
            ps1 = ctx.enter_context(tc.tile_pool(name="ps1", bufs=1,
                                                 space="PSUM"))
            pst = ctx.enter_context(tc.tile_pool(name="pst", bufs=2,
                                                 space="PSUM"))

            qt_sb = consts.tile([D, TQL], f32, tag="qt")
            vt_sb = consts.tile([D, TV], f32, tag="vt")
            vnp_sb = consts.tile([128, TV], f32, tag="vnp")
            w1_sb = consts.tile([D, U], f32, tag="w1")
            w2_sb = consts.tile([D, U], f32, tag="w2")
            sig_sb = consts.tile([U, 2 * TQL - 1], sdt, tag="sig")
            mka_sb = consts.tile([1, TV], f32, tag="mka")
            ones_sb = consts.tile([1, TQL], f32, tag="ones")
            id_sb = consts.tile([128, 128], f32, tag="id")
            wqT_sb = consts.tile([U, TQL], f32, tag="wqT")

            nc.sync.dma_start(qt_sb[:], qt[:])
            nc.sync.dma_start(vt_sb[:], vt[:])
            nc.sync.dma_start(vnp_sb[:], vnp[:])
            nc.sync.dma_start(w1_sb[:], w1[:])
            nc.sync.dma_start(w2_sb[:], w2[:])
            nc.sync.dma_start(sig_sb[:], sige[:])
            nc.sync.dma_start(mka_sb[:], mka[:])
            nc.sync.dma_start(ones_sb[:], ones1[:])
            nc.sync.dma_start(id_sb[:], ident[:])

            # wqT = W1.T @ qT  -> copy to SBUF (ACT bias source)
            wq_ps = ps1.tile([U, TQL], f32, tag="wq")
            nc.tensor.matmul(wq_ps[:], lhsT=w1_sb[:], rhs=qt_sb[:])
            nc.vector.tensor_copy(wqT_sb[:], wq_ps[:])

            # wkT = W2.T @ vT  -> stays in PSUM (ACT reads PSUM cheaper)
            wk_ps = ps1.tile([U, TV], f32, tag="wk")
            for j in range(NJ):
                nc.tensor.matmul(wk_ps[:, j * 512:(j + 1) * 512],
                                 lhsT=w2_sb[:],
                                 rhs=vt_sb[:, j * 512:(j + 1) * 512])

            scores_ps = ps1.tile([TQL, TV], f32, tag="scores")
            for q in range(TQL):
                g = gpool.tile([U, TV], sdt, tag="g")
                nc.scalar.activation(g[:], wk_ps[:], AFT.Tanh,
                                     bias=wqT_sb[:, q:q + 1])
                lw = sig_sb[:, TQL - 1 - q: 2 * TQL - 1 - q]
                for j in range(NJ):
                    nc.tensor.matmul(
                        scores_ps[:, j * 512:(j + 1) * 512],
                        lhsT=lw,
                        rhs=g[:, j * 512:(j + 1) * 512],
                        start=(q == 0), stop=False)

            # additive mask: scores[m, v] += maskadd[v] for every row m
            for j in range(NJ):
                nc.tensor.matmul(scores_ps[:, j * 512:(j + 1) * 512],
                                 lhsT=ones_sb[:],
                                 rhs=mka_sb[:, j * 512:(j + 1) * 512],
                                 start=False, stop=True)

            # exp + fused row-sum (no max subtraction needed: |scores| <~ 15)
            exp_sb = smp.tile([TQL, TV], f32, tag="exp")
            ssum = smp.tile([TQL, 1], f32, tag="ssum")
            rins = smp.tile([TQL, 1], f32, tag="rins")
            nc.scalar.activation(exp_sb[:], scores_ps[:], AFT.Exp,
                                 accum_out=ssum[:])
            nc.vector.reciprocal(rins[:], ssum[:])

            # ctx = softmax @ v  (transpose exp chunks, accumulate matmuls)
            ctx_ps = ps1.tile([TQL, D], f32, tag="ctx")
            for k in range(NVC):
                tp = pst.tile([128, 128], f32, tag="tp")
                nc.tensor.transpose(tp[:], exp_sb[:, k * 128:(k + 1) * 128],
                                    id_sb[:])
                et = etp.tile([128, 128], f32, tag="et")
                nc.vector.tensor_copy(et[:], tp[:])
                nc.tensor.matmul(ctx_ps[:], lhsT=et[:],
                                 rhs=vnp_sb[:, k * 128:(k + 1) * 128],
                                 start=(k == 0), stop=(k == NVC - 1))

            out_sb = smp.tile([TQL, D], f32, tag="out")
            nc.vector.tensor_scalar_mul(out_sb[:], ctx_ps[:], rins[:])
            nc.sync.dma_start(out[:], out_sb[:])

    nc.compile()
    return nc


def get_nc():
    if "nc" not in _CACHE:
        _CACHE["nc"] = _build_nc()
    return _CACHE["nc"]


def prep_in_maps(query, value, mask, W1, W2, scale):
    query = np.asarray(query, dtype=np.float32)
    value = np.asarray(value, dtype=np.float32)
    mask = np.asarray(mask)
    W1 = np.ascontiguousarray(np.asarray(W1, dtype=np.float32))
    W2 = np.ascontiguousarray(np.asarray(W2, dtype=np.float32))
    scale = np.asarray(scale, dtype=np.float32)

    sige = np.zeros((U, 2 * TQL - 1), np.float32)
    sige[:, TQL - 1] = scale
    ident = np.eye(128, dtype=np.float32)
    ones1 = np.ones((1, TQL), np.float32)

    in_maps = []
    for c in range(NCORES):
        b, q0 = c // 2, (c % 2) * TQL
        vb = value[:, b, :]  # [TV, D]
        in_maps.append({
            "qt": np.ascontiguousarray(query[q0:q0 + TQL, b, :].T),
            "vt": np.ascontiguousarray(vb.T),
            "vnp": np.ascontiguousarray(
                vb.reshape(TV // 128, 128, D).transpose(1, 0, 2)
                .reshape(128, TV // 128 * D)),
            "w1": W1,
            "w2": W2,
            "sige": sige,
            "mka": np.ascontiguousarray(
                np.where(mask[:, b], 0.0, NEG_INF).astype(np.float32)[None]),
            "ones1": ones1,
            "ident": ident,
        })
    return in_maps


def run(query, value, mask, W1, W2, scale, trace=False):
    from concourse.bass_utils import run_bass_kernel_spmd

    nc = get_nc()
    in_maps = prep_in_maps(query, value, mask, W1, W2, scale)
    res = run_bass_kernel_spmd(nc, in_maps, list(range(NCORES)), trace=trace)
    out = np.empty((TQ, B, D), np.float32)
    for c in range(NCORES):
        b, q0 = c // 2, (c % 2) * TQL
        out[q0:q0 + TQL, b, :] = res.results[c]["out"]
    return out, res


def kernel(query, value, mask, W1, W2, scale):
    out, _ = run(query, value, mask, W1, W2, scale, trace=False)
    return out


# revision 6
# speedup vs baseline: 1.3220x; 1.3220x over previous
"""Bahdanau (additive) attention kernel for Trainium2, 8 NeuronCores.

Full-input contract: kernel(**inputs) takes the unsharded numpy inputs and
returns the full [TQ, B, D] output. Internally shards (batch, query-half)
across 8 cores (B=4 x 2 halves of Tq), runs a Bass/Tile kernel per core via
run_bass_kernel_spmd, and reassembles.

Sparsity: masked value positions contribute exactly 0 to the softmax
(score + -1e9 -> exp underflows to 0), so the host gathers only the valid
value positions per batch (mask is input data), pads to a common multiple of
128 (TVE), and the device program is compiled for that TVE (cached).

Math per core (b = batch, 128 local queries, TVE gathered value positions):
  wqT[u,q] = sum_d W1[d,u] q[q,d]          (PE matmul, fp32)
  wkT[u,v] = sum_d W2[d,u] v[v,d]          (PE matmul, fp32; kept in PSUM)
  g_q[u,v] = tanh(wkT[u,v] + wqT[u,q])     (ACT, per-partition bias = wqT[:,q])
  scores[q,v] = sum_u scale[u] g_q[u,v]    (PE matmul, float32r, sliding-window
                                            lhsT = sigma in column q, else 0)
  scores += padadd[v]                      (PE K=1 matmul: -1e9 on pad columns)
  e = exp(scores); ssum = rowsum(e)        (ACT with accum_out)
  ctx[q,d] = (1/ssum[q]) sum_v e[q,v] v[v,d]  (PE transpose + matmuls, DVE scale)
"""

import sys

if "/opt/trn_rl_repo" not in sys.path:
    sys.path.insert(0, "/opt/trn_rl_repo")

import numpy as np

TQ, TV, B, D, U = 256, 1024, 4, 128, 128
NCORES = 8
TQL = 128  # local queries per core (Tq=256 split in 2 per batch)
NEG_INF = -1e9

# Score-contraction matmul dtype: "f32r" (fast, ~1e-4 rel err) or "f32"
# (exact, 4 cyc/row -> PE becomes the bottleneck).
SCORE_DT = "f32r"

_CACHE = {}


def _bank_pieces(tve):
    """Split [0, tve) into PSUM-bank-aligned matmul slices (<=512 each)."""
    pieces = []
    a = 0
    while a < tve:
        n = min(512, tve - a)
        pieces.append((a, n))
        a += n
    return pieces


def _build_nc(tve):
    import concourse.bacc as bacc
    import concourse.mybir as mybir
    import concourse.tile as tile
    from contextlib import ExitStack

    f32 = mybir.dt.float32
    f32r = mybir.dt.float32r
    AFT = mybir.ActivationFunctionType

    nc = bacc.Bacc("TRN2", target_bir_lowering=False, debug=False,
                   num_devices=NCORES)

    sdt = f32r if SCORE_DT == "f32r" else f32

    qt = nc.dram_tensor("qt", [D, TQL], f32, kind="ExternalInput").ap()
    vt = nc.dram_tensor("vt", [D, tve], f32, kind="ExternalInput").ap()
    vnp = nc.dram_tensor("vnp", [128, tve], f32, kind="ExternalInput").ap()
    w1 = nc.dram_tensor("w1", [D, U], f32, kind="ExternalInput").ap()
    w2 = nc.dram_tensor("w2", [D, U], f32, kind="ExternalInput").ap()
    sige = nc.dram_tensor("sige", [U, 2 * TQL - 1], sdt,
                          kind="ExternalInput").ap()
    mka = nc.dram_tensor("mka", [1, tve], f32, kind="ExternalInput").ap()
    ones1 = nc.dram_tensor("ones1", [1, TQL], f32, kind="ExternalInput").ap()
    ident = nc.dram_tensor("ident", [128, 128], f32, kind="ExternalInput").ap()
    out = nc.dram_tensor("out", [TQL, D], f32, kind="ExternalOutput").ap()

    NVC = tve // 128
    pieces = _bank_pieces(tve)

    with tile.TileContext(nc) as tc:
        with ExitStack() as ctx:
            consts = ctx.enter_context(tc.tile_pool(name="consts", bufs=1))
            gpool = ctx.enter_context(tc.tile_pool(name="g", bufs=6))
            smp = ctx.enter_context(tc.tile_pool(name="sm", bufs=1))
            etp = ctx.enter_context(tc.tile_pool(name="et", bufs=2))
            ps1 = ctx.enter_context(tc.tile_pool(name="ps1", bufs=1,
                                                 space="PSUM"))
            pst = ctx.enter_context(tc.tile_pool(name="pst", bufs=2,
                                                 space="PSUM"))

            qt_sb = consts.tile([D, TQL], f32, tag="qt")
            vt_sb = consts.tile([D, tve], f32, tag="vt")
            vnp_sb = consts.tile([128, tve], f32, tag="vnp")
            w1_sb = consts.tile([D, U], f32, tag="w1")
            w2_sb = consts.tile([D, U], f32, tag="w2")
            sig_sb = consts.tile([U, 2 * TQL - 1], sdt, tag="sig")
            mka_sb = consts.tile([1, tve], f32, tag="mka")
            ones_sb = consts.tile([1, TQL], f32, tag="ones")
            id_sb = consts.tile([128, 128], f32, tag="id")
            wqT_sb = consts.tile([U, TQL], f32, tag="wqT")

            # loads ordered so the critical chain (w2, vt -> wk) starts first
            nc.sync.dma_start(w2_sb[:], w2[:])
            nc.sync.dma_start(vt_sb[:], vt[:])
            nc.sync.dma_start(w1_sb[:], w1[:])
            nc.sync.dma_start(qt_sb[:], qt[:])
            nc.sync.dma_start(sig_sb[:], sige[:])
            nc.sync.dma_start(vnp_sb[:], vnp[:])
            nc.sync.dma_start(mka_sb[:], mka[:])
            nc.sync.dma_start(ones_sb[:], ones1[:])
            nc.sync.dma_start(id_sb[:], ident[:])

            # wkT = W2.T @ vT  -> stays in PSUM (ACT reads PSUM cheaper)
            wk_ps = ps1.tile([U, tve], f32, tag="wk")
            for a, n in pieces:
                nc.tensor.matmul(wk_ps[:, a:a + n], lhsT=w2_sb[:],
                                 rhs=vt_sb[:, a:a + n])

            # wqT = W1.T @ qT  -> copy to SBUF (ACT bias source)
            wq_ps = ps1.tile([U, TQL], f32, tag="wq")
            nc.tensor.matmul(wq_ps[:], lhsT=w1_sb[:], rhs=qt_sb[:])
            nc.vector.tensor_copy(wqT_sb[:], wq_ps[:])

            scores_ps = ps1.tile([TQL, tve], f32, tag="scores")
            for q in range(TQL):
                g = gpool.tile([U, tve], sdt, tag="g")
                nc.scalar.activation(g[:], wk_ps[:], AFT.Tanh,
                                     bias=wqT_sb[:, q:q + 1])
                lw = sig_sb[:, TQL - 1 - q: 2 * TQL - 1 - q]
                for a, n in pieces:
                    nc.tensor.matmul(scores_ps[:, a:a + n],
                                     lhsT=lw, rhs=g[:, a:a + n],
                                     start=(q == 0), stop=False)

            # pad/mask add: scores[m, v] += mka[v] for every row m
            for a, n in pieces:
                nc.tensor.matmul(scores_ps[:, a:a + n],
                                 lhsT=ones_sb[:], rhs=mka_sb[:, a:a + n],
                                 start=False, stop=True)

            # exp + fused row-sum (no max subtraction needed: |scores| <~ 15)
            exp_sb = smp.tile([TQL, tve], f32, tag="exp")
            ssum = smp.tile([TQL, 1], f32, tag="ssum")
            rins = smp.tile([TQL, 1], f32, tag="rins")
            nc.scalar.activation(exp_sb[:], scores_ps[:], AFT.Exp,
                                 accum_out=ssum[:])
            nc.vector.reciprocal(rins[:], ssum[:])

            # ctx = softmax @ v  (transpose exp chunks, accumulate matmuls)
            ctx_ps = ps1.tile([TQL, D], f32, tag="ctx")
            for k in range(NVC):
                tp = pst.tile([128, 128], f32, tag="tp")
                nc.tensor.transpose(tp[:], exp_sb[:, k * 128:(k + 1) * 128],
                                    id_sb[:])
                et = etp.tile([128, 128], f32, tag="et")
                nc.vector.tensor_copy(et[:], tp[:])
                nc.tensor.matmul(ctx_ps[:], lhsT=et[:],
                                 rhs=vnp_sb[:, k * 128:(k + 1) * 128],
                                 start=(k == 0), stop=(k == NVC - 1))

            out_sb = smp.tile([TQL, D], f32, tag="out")
            nc.vector.tensor_scalar_mul(out_sb[:], ctx_ps[:], rins[:])
            nc.sync.dma_start(out[:], out_sb[:])

    nc.compile()
    return nc


def get_nc(tve=TV):
    key = ("nc", tve)
    if key not in _CACHE:
        _CACHE[key] = _build_nc(tve)
    return _CACHE[key]


def prep_in_maps(query, value, mask, W1, W2, scale):
    """Gather valid value positions per batch; returns (in_maps, tve)."""
    query = np.asarray(query, dtype=np.float32)
    value = np.asarray(value, dtype=np.float32)
    mask = np.asarray(mask)
    W1 = np.ascontiguousarray(np.asarray(W1, dtype=np.float32))
    W2 = np.ascontiguousarray(np.asarray(W2, dtype=np.float32))
    scale = np.asarray(scale, dtype=np.float32)

    idxs = [np.nonzero(mask[:, b])[0] for b in range(B)]
    nv_max = max(1, max(len(ix) for ix in idxs))
    tve = min(TV, -(-nv_max // 128) * 128)

    sige = np.zeros((U, 2 * TQL - 1), np.float32)
    sige[:, TQL - 1] = scale
    ident = np.eye(128, dtype=np.float32)
    ones1 = np.ones((1, TQL), np.float32)

    in_maps = []
    for c in range(NCORES):
        b, q0 = c // 2, (c % 2) * TQL
        ix = idxs[b]
        nv = len(ix)
        vg = np.zeros((tve, D), np.float32)
        vg[:nv] = value[ix, b, :]
        mka = np.zeros((1, tve), np.float32)
        mka[0, nv:] = NEG_INF
        in_maps.append({
            "qt": np.ascontiguousarray(query[q0:q0 + TQL, b, :].T),
            "vt": np.ascontiguousarray(vg.T),
            "vnp": np.ascontiguousarray(
                vg.reshape(tve // 128, 128, D).transpose(1, 0, 2)
                .reshape(128, tve // 128 * D)),
            "w1": W1,
            "w2": W2,
            "sige": sige,
            "mka": mka,
            "ones1": ones1,
            "ident": ident,
        })
    return in_maps, tve


def run(query, value, mask, W1, W2, scale, trace=False):
    from concourse.bass_utils import run_bass_kernel_spmd

    in_maps, tve = prep_in_maps(query, value, mask, W1, W2, scale)
    nc = get_nc(tve)
    res = run_bass_kernel_spmd(nc, in_maps, list(range(NCORES)), trace=trace)
    out = np.empty((TQ, B, D), np.float32)
    for c in range(NCORES):
        b, q0 = c // 2, (c % 2) * TQL
        out[q0:q0 + TQL, b, :] = res.results[c]["out"]
    return out, res


def kernel(query, value, mask, W1, W2, scale):
    out, _ = run(query, value, mask, W1, W2, scale, trace=False)
    return out


# revision 10
# speedup vs baseline: 1.3677x; 1.0346x over previous
"""Bahdanau (additive) attention kernel for Trainium2, 8 NeuronCores.

Full-input contract: kernel(**inputs) takes the unsharded numpy inputs and
returns the full [TQ, B, D] output. Internally shards (batch, query-half)
across 8 cores (B=4 x 2 halves of Tq), runs a Bass/Tile kernel per core via
run_bass_kernel_spmd, and reassembles.

Sparsity: masked value positions contribute exactly 0 to the softmax
(score + -1e9 -> exp underflows to 0), so the host gathers only the valid
value positions per batch (mask is input data), pads to a common multiple of
128 (TVE), and the device program is compiled for that TVE (cached).

Math per core (b = batch, 128 local queries, TVE gathered value positions):
  wqT[u,q] = sum_d W1[d,u] q[q,d]          (PE matmul, fp32)
  wkT[u,v] = sum_d W2[d,u] v[v,d]          (PE matmul, fp32; kept in PSUM)
  g_q[u,v] = tanh(wkT[u,v] + wqT[u,q])     (ACT, per-partition bias = wqT[:,q])
  scores[q,v] = sum_u scale[u] g_q[u,v]    (PE matmul, float32r, sliding-window
                                            lhsT = sigma in column q, else 0)
  scores += padadd[v]                      (PE K=1 matmul: -1e9 on pad columns)
  e = exp(scores); ssum = rowsum(e)        (ACT with accum_out)
  ctx[q,d] = (1/ssum[q]) sum_v e[q,v] v[v,d]  (PE transpose + matmuls, DVE scale)
"""

import sys

if "/opt/trn_rl_repo" not in sys.path:
    sys.path.insert(0, "/opt/trn_rl_repo")

import numpy as np

TQ, TV, B, D, U = 256, 1024, 4, 128, 128
NCORES = 8
TQL = 128  # local queries per core (Tq=256 split in 2 per batch)
NEG_INF = -1e9

# Score-contraction matmul dtype:
#   "f32r" — reduced-precision fp32 (~1e-4 rel err), 2 cyc/row
#   "bf16" — bfloat16 (~3e-3 rel err), 1 cyc/row + fast weight load
#   "f32"  — exact, 4 cyc/row (PE becomes the bottleneck)
SCORE_DT = "bf16"

_CACHE = {}


def _bank_pieces(tve):
    """Split [0, tve) into PSUM-bank-aligned matmul slices (<=512 each)."""
    pieces = []
    a = 0
    while a < tve:
        n = min(512, tve - a)
        pieces.append((a, n))
        a += n
    return pieces


def _build_nc(tve):
    import concourse.bacc as bacc
    import concourse.mybir as mybir
    import concourse.tile as tile
    from contextlib import ExitStack

    f32 = mybir.dt.float32
    f32r = mybir.dt.float32r
    AFT = mybir.ActivationFunctionType

    nc = bacc.Bacc("TRN2", target_bir_lowering=False, debug=False,
                   num_devices=NCORES)

    sdt = {"f32r": f32r, "bf16": mybir.dt.bfloat16, "f32": f32}[SCORE_DT]

    qt = nc.dram_tensor("qt", [D, TQL], f32, kind="ExternalInput").ap()
    vt = nc.dram_tensor("vt", [D, tve], f32, kind="ExternalInput").ap()
    vnp = nc.dram_tensor("vnp", [128, tve], f32, kind="ExternalInput").ap()
    w1 = nc.dram_tensor("w1", [D, U], f32, kind="ExternalInput").ap()
    w2 = nc.dram_tensor("w2", [D, U], f32, kind="ExternalInput").ap()
    sige = nc.dram_tensor("sige", [U, 2 * TQL - 1], sdt,
                          kind="ExternalInput").ap()
    mka = nc.dram_tensor("mka", [1, tve], f32, kind="ExternalInput").ap()
    ones1 = nc.dram_tensor("ones1", [1, TQL], f32, kind="ExternalInput").ap()
    ident = nc.dram_tensor("ident", [128, 128], f32, kind="ExternalInput").ap()
    out = nc.dram_tensor("out", [TQL, D], f32, kind="ExternalOutput").ap()

    NVC = tve // 128
    pieces = _bank_pieces(tve)

    with tile.TileContext(nc) as tc:
        with ExitStack() as ctx:
            consts = ctx.enter_context(tc.tile_pool(name="consts", bufs=1))
            gpool = ctx.enter_context(tc.tile_pool(name="g", bufs=6))
            smp = ctx.enter_context(tc.tile_pool(name="sm", bufs=1))
            etp = ctx.enter_context(tc.tile_pool(name="et", bufs=2))
            ps1 = ctx.enter_context(tc.tile_pool(name="ps1", bufs=1,
                                                 space="PSUM"))
            pst = ctx.enter_context(tc.tile_pool(name="pst", bufs=2,
                                                 space="PSUM"))

            qt_sb = consts.tile([D, TQL], f32, tag="qt")
            vt_sb = consts.tile([D, tve], f32, tag="vt")
            vnp_sb = consts.tile([128, tve], f32, tag="vnp")
            w1_sb = consts.tile([D, U], f32, tag="w1")
            w2_sb = consts.tile([D, U], f32, tag="w2")
            sig_sb = consts.tile([U, 2 * TQL - 1], sdt, tag="sig")
            mka_sb = consts.tile([1, tve], f32, tag="mka")
            ones_sb = consts.tile([1, TQL], f32, tag="ones")
            id_sb = consts.tile([128, 128], f32, tag="id")
            wqT_sb = consts.tile([U, TQL], f32, tag="wqT")

            # preload the exp/tanh ACT table set during the input DMAs
            warm_in = consts.tile([128, 1], f32, tag="warm_in")
            warm_out = consts.tile([128, 1], f32, tag="warm_out")
            nc.gpsimd.memset(warm_in[:], 0.0)
            nc.scalar.activation(warm_out[:], warm_in[:], AFT.Tanh)

            # loads ordered so the critical chain (w2, vt -> wk) starts first
            nc.sync.dma_start(w2_sb[:], w2[:])
            nc.sync.dma_start(vt_sb[:], vt[:])
            nc.sync.dma_start(w1_sb[:], w1[:])
            nc.sync.dma_start(qt_sb[:], qt[:])
            nc.sync.dma_start(sig_sb[:], sige[:])
            nc.sync.dma_start(vnp_sb[:], vnp[:])
            nc.sync.dma_start(mka_sb[:], mka[:])
            nc.sync.dma_start(ones_sb[:], ones1[:])
            nc.sync.dma_start(id_sb[:], ident[:])

            # wkT = W2.T @ vT  -> stays in PSUM (ACT reads PSUM cheaper)
            wk_ps = ps1.tile([U, tve], f32, tag="wk")
            for a, n in pieces:
                nc.tensor.matmul(wk_ps[:, a:a + n], lhsT=w2_sb[:],
                                 rhs=vt_sb[:, a:a + n])

            # wqT = W1.T @ qT  -> copy to SBUF (ACT bias source)
            wq_ps = ps1.tile([U, TQL], f32, tag="wq")
            nc.tensor.matmul(wq_ps[:], lhsT=w1_sb[:], rhs=qt_sb[:])
            nc.vector.tensor_copy(wqT_sb[:], wq_ps[:])

            scores_ps = ps1.tile([TQL, tve], f32, tag="scores")
            # pad/mask add opens the accumulation: scores[m, v] = mka[v]
            for a, n in pieces:
                nc.tensor.matmul(scores_ps[:, a:a + n],
                                 lhsT=ones_sb[:], rhs=mka_sb[:, a:a + n],
                                 start=True, stop=False)
            for q in range(TQL):
                g = gpool.tile([U, tve], sdt, tag="g")
                nc.scalar.activation(g[:], wk_ps[:], AFT.Tanh,
                                     bias=wqT_sb[:, q:q + 1])
                lw = sig_sb[:, TQL - 1 - q: 2 * TQL - 1 - q]
                for a, n in pieces:
                    nc.tensor.matmul(scores_ps[:, a:a + n],
                                     lhsT=lw, rhs=g[:, a:a + n],
                                     start=False, stop=(q == TQL - 1))

            # exp + fused row-sum (no max subtraction needed: |scores| <~ 15)
            exp_sb = smp.tile([TQL, tve], f32, tag="exp")
            ssum = smp.tile([TQL, 1], f32, tag="ssum")
            rins = smp.tile([TQL, 1], f32, tag="rins")
            nc.scalar.activation(exp_sb[:], scores_ps[:], AFT.Exp,
                                 accum_out=ssum[:])
            nc.vector.reciprocal(rins[:], ssum[:])

            # ctx = softmax @ v  (transpose exp chunks, accumulate matmuls)
            ctx_ps = ps1.tile([TQL, D], f32, tag="ctx")
            for k in range(NVC):
                tp = pst.tile([128, 128], f32, tag="tp")
                nc.tensor.transpose(tp[:], exp_sb[:, k * 128:(k + 1) * 128],
                                    id_sb[:])
                et = etp.tile([128, 128], f32, tag="et")
                nc.vector.tensor_copy(et[:], tp[:])
                nc.tensor.matmul(ctx_ps[:], lhsT=et[:],
                                 rhs=vnp_sb[:, k * 128:(k + 1) * 128],
                                 start=(k == 0), stop=(k == NVC - 1))

            out_sb = smp.tile([TQL, D], f32, tag="out")
            nc.vector.tensor_scalar_mul(out_sb[:], ctx_ps[:], rins[:])
            nc.sync.dma_start(out[:], out_sb[:])

    nc.compile()
    return nc


def get_nc(tve=TV):
    key = ("nc", tve)
    if key not in _CACHE:
        _CACHE[key] = _build_nc(tve)
    return _CACHE[key]


def prep_in_maps(query, value, mask, W1, W2, scale):
    """Gather valid value positions per batch; returns (in_maps, tve)."""
    query = np.asarray(query, dtype=np.float32)
    value = np.asarray(value, dtype=np.float32)
    mask = np.asarray(mask)
    W1 = np.ascontiguousarray(np.asarray(W1, dtype=np.float32))
    W2 = np.ascontiguousarray(np.asarray(W2, dtype=np.float32))
    scale = np.asarray(scale, dtype=np.float32)

    idxs = [np.nonzero(mask[:, b])[0] for b in range(B)]
    nv_max = max(1, max(len(ix) for ix in idxs))
    tve = min(TV, -(-nv_max // 128) * 128)

    if SCORE_DT == "bf16":
        import ml_dtypes
        sdt_np = np.dtype(ml_dtypes.bfloat16)
    else:
        sdt_np = np.float32
    sige = np.zeros((U, 2 * TQL - 1), sdt_np)
    sige[:, TQL - 1] = scale.astype(sdt_np)
    ident = np.eye(128, dtype=np.float32)
    ones1 = np.ones((1, TQL), np.float32)

    in_maps = []
    for c in range(NCORES):
        b, q0 = c // 2, (c % 2) * TQL
        ix = idxs[b]
        nv = len(ix)
        vg = np.zeros((tve, D), np.float32)
        vg[:nv] = value[ix, b, :]
        mka = np.zeros((1, tve), np.float32)
        mka[0, nv:] = NEG_INF
        in_maps.append({
            "qt": np.ascontiguousarray(query[q0:q0 + TQL, b, :].T),
            "vt": np.ascontiguousarray(vg.T),
            "vnp": np.ascontiguousarray(
                vg.reshape(tve // 128, 128, D).transpose(1, 0, 2)
                .reshape(128, tve // 128 * D)),
            "w1": W1,
            "w2": W2,
            "sige": sige,
            "mka": mka,
            "ones1": ones1,
            "ident": ident,
        })
    return in_maps, tve


def run(query, value, mask, W1, W2, scale, trace=False):
    from concourse.bass_utils import run_bass_kernel_spmd

    in_maps, tve = prep_in_maps(query, value, mask, W1, W2, scale)
    nc = get_nc(tve)
    res = run_bass_kernel_spmd(nc, in_maps, list(range(NCORES)), trace=trace)
    out = np.empty((TQ, B, D), np.float32)
    for c in range(NCORES):
        b, q0 = c // 2, (c % 2) * TQL
        out[q0:q0 + TQL, b, :] = res.results[c]["out"]
    return out, res


def kernel(query, value, mask, W1, W2, scale):
    out, _ = run(query, value, mask, W1, W2, scale, trace=False)
    return out
